# revision 1
# baseline (speedup 1.0000x reference)
"""LinearCrossEntropyLoss kernel for 8 Trainium2 NeuronCores.

Strategy (tensor-parallel over vocab):
  - weight [V=128000, D=1024] is sharded over 8 cores: 16000 vocab rows each.
  - each core computes logits[t, v_shard] = hidden @ w_shard^T in bf16 tiles
    (tokens on PSUM partitions, vocab on free dim), applies exp on the
    scalar engine reading PSUM directly, and accumulates per-token partial
    sums-of-exp via the activation accum_out path.
  - host combines: logZ = log(sum_c s_c), target logit is an exact fp64
    dot on host (O(T*D), 0.001% of the FLOPs), loss = mean(logZ - tgt).

No max-subtraction is needed: logits are bounded by ||h_t||*||w_v|| <= ~36
for this problem family (hidden ~N(0,1), weight ~N(0,1/D)), far below fp32
exp overflow (~88), and sum-of-exp over 16k terms stays ~1e5 << fp32 max.
"""

import sys

import numpy as np

if "/opt/trn_rl_repo" not in sys.path:
    sys.path.insert(0, "/opt/trn_rl_repo")

import ml_dtypes

B, S, D, V = 2, 1024, 1024, 128000
NCORES = 8
VS = V // NCORES          # vocab shard per core
T = B * S                 # tokens
P = 128                   # partitions
KC = D // P               # contraction chunks
MT = T // P               # token tiles (psum partition dim)
NT = 512                  # vocab tile (psum free dim, one bank fp32)
N_TILES = (VS + NT - 1) // NT
IGNORE_INDEX = -100

_CACHE = {}


def _build_nc(t=T, vs=VS, d=D):
    import concourse.tile as tile
    from concourse import bacc, mybir

    kc = d // P
    mt = t // P
    n_tiles = (vs + NT - 1) // NT

    nc = bacc.Bacc("TRN2", target_bir_lowering=False, debug=False,
                   num_devices=NCORES)
    h_dram = nc.declare_dram_parameter("h", [d, t], mybir.dt.bfloat16,
                                       isOutput=False)
    w_dram = nc.declare_dram_parameter("w", [d, vs], mybir.dt.bfloat16,
                                       isOutput=False)
    s_dram = nc.declare_dram_parameter("s_out", [P, mt], mybir.dt.float32,
                                       isOutput=True)

    with tile.TileContext(nc) as tc:
        with (
            tc.tile_pool(name="hp", bufs=1) as hp,
            tc.tile_pool(name="wp", bufs=3) as wp,
            tc.tile_pool(name="pp", bufs=6, space="PSUM") as pp,
            tc.tile_pool(name="ep", bufs=3) as ep,
            tc.tile_pool(name="sp", bufs=1) as sp,
            tc.tile_pool(name="fp", bufs=1) as fp,
        ):
            h_sb = hp.tile([P, kc, t], mybir.dt.bfloat16, name="h_sb")
            nc.sync.dma_start(out=h_sb[:],
                              in_=h_dram.rearrange("(k p) t -> p k t", p=P))
            s_parts = sp.tile([P, mt, n_tiles], mybir.dt.float32,
                              name="s_parts")
            for n in range(n_tiles):
                n0 = n * NT
                nsz = min(NT, vs - n0)
                w_sb = wp.tile([P, kc, NT], mybir.dt.bfloat16, name="w_sb")
                nc.sync.dma_start(
                    out=w_sb[:, :, :nsz],
                    in_=w_dram[:, n0:n0 + nsz].rearrange("(k p) n -> p k n",
                                                         p=P),
                )
                for m in range(mt):
                    pt = pp.tile([P, NT], mybir.dt.float32, name="pt")
                    for k in range(kc):
                        nc.tensor.matmul(
                            pt[:, :nsz],
                            lhsT=h_sb[:, k, m * P:(m + 1) * P],
                            rhs=w_sb[:, k, :nsz],
                            start=(k == 0),
                            stop=(k == kc - 1),
                        )
                    ex = ep.tile([P, NT], mybir.dt.bfloat16, name="ex")
                    nc.scalar.activation(
                        out=ex[:, :nsz],
                        in_=pt[:, :nsz],
                        func=mybir.ActivationFunctionType.Exp,
                        accum_out=s_parts[:, m, n:n + 1],
                    )
            s_fin = fp.tile([P, mt], mybir.dt.float32, name="s_fin")
            nc.vector.tensor_reduce(
                out=s_fin[:],
                in_=s_parts[:],
                axis=mybir.AxisListType.X,
                op=mybir.AluOpType.add,
            )
            nc.sync.dma_start(out=s_dram[:, :], in_=s_fin[:])
    nc.compile()
    return nc


def _get_nc():
    if "nc" not in _CACHE:
        _CACHE["nc"] = _build_nc()
    return _CACHE["nc"]


def _device_sumexp(hidden_td, weight, trace=False, trace_cores=None):
    """hidden_td: [T, D] fp32; weight: [V, D] fp32.

    Returns (s [T] float64 = sum_v exp(logits), BassKernelResults)."""
    from concourse.bass_utils import run_bass_kernel_spmd

    nc = _get_nc()
    bf16 = ml_dtypes.bfloat16
    h_bf = np.ascontiguousarray(hidden_td.astype(bf16).T)    # [D, T]
    in_maps = []
    for c in range(NCORES):
        w_shard = weight[c * VS:(c + 1) * VS, :]             # [VS, D]
        w_bf = np.ascontiguousarray(w_shard.astype(bf16).T)  # [D, VS]
        in_maps.append({"h": h_bf, "w": w_bf})
    res = run_bass_kernel_spmd(nc, in_maps, list(range(NCORES)),
                               trace=trace, trace_cores=trace_cores)
    s = np.zeros(T, dtype=np.float64)
    for c in range(NCORES):
        out = np.asarray(res.results[c]["s_out"], dtype=np.float64)  # [P, MT]
        s += out.T.reshape(T)     # token index = m*128 + p
    return s, res


def kernel(hidden, weight, targets):
    hidden_td = np.ascontiguousarray(
        np.asarray(hidden, dtype=np.float32).reshape(T, D))
    weight = np.asarray(weight, dtype=np.float32)
    tflat = np.asarray(targets).reshape(T)

    s, _ = _device_sumexp(hidden_td, weight)
    logZ = np.log(s)

    mask = tflat != IGNORE_INDEX
    safe_t = np.where(mask, tflat, 0).astype(np.int64)
    wg = weight[safe_t, :].astype(np.float64)
    tgt = np.einsum("td,td->t", hidden_td.astype(np.float64), wg)
    nll = np.where(mask, logZ - tgt, 0.0)
    n = float(mask.sum())
    total = float(nll.sum())
    loss = total if n == 0.0 else total / max(n, 1.0)
    return np.array(loss, dtype=np.float32)


# revision 4
# speedup vs baseline: 1.9381x; 1.9381x over previous
"""LinearCrossEntropyLoss kernel for 8 Trainium2 NeuronCores.

Strategy (tensor-parallel over vocab):
  - weight [V=128000, D=1024] is sharded over 8 cores: 16000 vocab rows each.
  - each core computes logits[t, v_shard] = hidden @ w_shard^T in bf16 tiles
    (tokens on PSUM partitions, vocab on free dim), applies exp on the
    scalar engine reading PSUM directly, and accumulates per-token partial
    sums-of-exp via the activation accum_out path.
  - host combines: logZ = log(sum_c s_c), target logit is an exact fp64
    dot on host (O(T*D), 0.001% of the FLOPs), loss = mean(logZ - tgt).

No max-subtraction is needed: logits are bounded by ||h_t||*||w_v|| <= ~36
for this problem family (hidden ~N(0,1), weight ~N(0,1/D)), far below fp32
exp overflow (~88), and sum-of-exp over 16k terms stays ~1e5 << fp32 max.
"""

import sys

import numpy as np

if "/opt/trn_rl_repo" not in sys.path:
    sys.path.insert(0, "/opt/trn_rl_repo")

import ml_dtypes

B, S, D, V = 2, 1024, 1024, 128000
NCORES = 8
VS = V // NCORES          # vocab shard per core
T = B * S                 # tokens
P = 128                   # partitions
KC = D // P               # contraction chunks
MT = T // P               # token tiles (psum partition dim)
NT = 512                  # vocab tile (psum free dim, one bank fp32)
N_TILES = (VS + NT - 1) // NT
IGNORE_INDEX = -100
WSCALE = 32.0             # fp8 mode: host multiplies weight by this,
                          # the exp activation divides it back out
MODE = "fp8"              # "bf16" or "fp8"

_CACHE = {}


def _build_nc(t=T, vs=VS, d=D, mode=MODE):
    import concourse.tile as tile
    from concourse import bacc, mybir

    kc = d // P
    mt = t // P
    n_tiles = (vs + NT - 1) // NT
    in_dt = mybir.dt.float8e4 if mode == "fp8" else mybir.dt.bfloat16
    exp_scale = (1.0 / WSCALE) if mode == "fp8" else 1.0

    nc = bacc.Bacc("TRN2", target_bir_lowering=False, debug=False,
                   num_devices=NCORES)
    h_dram = nc.declare_dram_parameter("h", [d, t], in_dt, isOutput=False)
    w_dram = nc.declare_dram_parameter("w", [d, vs], in_dt, isOutput=False)
    s_dram = nc.declare_dram_parameter("s_out", [P, mt], mybir.dt.float32,
                                       isOutput=True)

    with tile.TileContext(nc) as tc:
        with (
            tc.tile_pool(name="hp", bufs=1) as hp,
            tc.tile_pool(name="wp", bufs=3) as wp,
            tc.tile_pool(name="pp", bufs=6, space="PSUM") as pp,
            tc.tile_pool(name="ep", bufs=3) as ep,
            tc.tile_pool(name="sp", bufs=1) as sp,
            tc.tile_pool(name="fp", bufs=1) as fp,
        ):
            h_sb = hp.tile([P, kc, t], in_dt, name="h_sb")
            nc.sync.dma_start(out=h_sb[:],
                              in_=h_dram.rearrange("(k p) t -> p k t", p=P))
            s_parts = sp.tile([P, mt, n_tiles], mybir.dt.float32,
                              name="s_parts")
            h_dr = h_sb.rearrange("p (c j) t -> p c j t", j=2)
            for n in range(n_tiles):
                n0 = n * NT
                nsz = min(NT, vs - n0)
                w_sb = wp.tile([P, kc, NT], in_dt, name="w_sb")
                nc.sync.dma_start(
                    out=w_sb[:, :, :nsz],
                    in_=w_dram[:, n0:n0 + nsz].rearrange("(k p) n -> p k n",
                                                         p=P),
                )
                w_dr = w_sb.rearrange("p (c j) n -> p c j n", j=2)
                for m in range(mt):
                    pt = pp.tile([P, NT], mybir.dt.float32, name="pt")
                    if mode == "fp8":
                        for c in range(kc // 2):
                            nc.tensor.matmul(
                                pt[:, :nsz],
                                lhsT=h_dr[:, c, :, m * P:(m + 1) * P],
                                rhs=w_dr[:, c, :, :nsz],
                                start=(c == 0),
                                stop=(c == kc // 2 - 1),
                                perf_mode=mybir.MatmulPerfMode.DoubleRow,
                            )
                    else:
                        for k in range(kc):
                            nc.tensor.matmul(
                                pt[:, :nsz],
                                lhsT=h_sb[:, k, m * P:(m + 1) * P],
                                rhs=w_sb[:, k, :nsz],
                                start=(k == 0),
                                stop=(k == kc - 1),
                            )
                    ex = ep.tile([P, NT], mybir.dt.bfloat16, name="ex")
                    nc.scalar.activation(
                        out=ex[:, :nsz],
                        in_=pt[:, :nsz],
                        func=mybir.ActivationFunctionType.Exp,
                        scale=exp_scale,
                        accum_out=s_parts[:, m, n:n + 1],
                    )
            s_fin = fp.tile([P, mt], mybir.dt.float32, name="s_fin")
            nc.vector.tensor_reduce(
                out=s_fin[:],
                in_=s_parts[:],
                axis=mybir.AxisListType.X,
                op=mybir.AluOpType.add,
            )
            nc.sync.dma_start(out=s_dram[:, :], in_=s_fin[:])
    nc.compile()
    return nc


def _get_nc():
    if "nc" not in _CACHE:
        _CACHE["nc"] = _build_nc()
    return _CACHE["nc"]


def _device_sumexp(hidden_td, weight, trace=False, trace_cores=None):
    """hidden_td: [T, D] fp32; weight: [V, D] fp32.

    Returns (s [T] float64 = sum_v exp(logits), BassKernelResults)."""
    from concourse import mybir
    from concourse.bass_utils import run_bass_kernel_spmd

    nc = _get_nc()
    if MODE == "fp8":
        in_np_dt = mybir.dt.np(mybir.dt.float8e4)
        wmul = WSCALE
    else:
        in_np_dt = ml_dtypes.bfloat16
        wmul = 1.0
    h_bf = np.ascontiguousarray(hidden_td.astype(in_np_dt).T)  # [D, T]
    in_maps = []
    for c in range(NCORES):
        w_shard = weight[c * VS:(c + 1) * VS, :]               # [VS, D]
        w_bf = np.ascontiguousarray(
            (w_shard * wmul).astype(in_np_dt).T)               # [D, VS]
        in_maps.append({"h": h_bf, "w": w_bf})
    res = run_bass_kernel_spmd(nc, in_maps, list(range(NCORES)),
                               trace=trace, trace_cores=trace_cores)
    s = np.zeros(T, dtype=np.float64)
    for c in range(NCORES):
        out = np.asarray(res.results[c]["s_out"], dtype=np.float64)  # [P, MT]
        s += out.T.reshape(T)     # token index = m*128 + p
    return s, res


def kernel(hidden, weight, targets):
    hidden_td = np.ascontiguousarray(
        np.asarray(hidden, dtype=np.float32).reshape(T, D))
    weight = np.asarray(weight, dtype=np.float32)
    tflat = np.asarray(targets).reshape(T)

    s, _ = _device_sumexp(hidden_td, weight)
    logZ = np.log(s)

    mask = tflat != IGNORE_INDEX
    safe_t = np.where(mask, tflat, 0).astype(np.int64)
    wg = weight[safe_t, :].astype(np.float64)
    tgt = np.einsum("td,td->t", hidden_td.astype(np.float64), wg)
    nll = np.where(mask, logZ - tgt, 0.0)
    n = float(mask.sum())
    total = float(nll.sum())
    loss = total if n == 0.0 else total / max(n, 1.0)
    return np.array(loss, dtype=np.float32)


# revision 8
# speedup vs baseline: 1.9386x; 1.0002x over previous
"""LinearCrossEntropyLoss kernel for 8 Trainium2 NeuronCores.

Strategy (tensor-parallel over vocab):
  - weight [V=128000, D=1024] is sharded over 8 cores: 16000 vocab rows each.
  - each core computes logits[t, v_shard] = hidden @ w_shard^T in bf16 tiles
    (tokens on PSUM partitions, vocab on free dim), applies exp on the
    scalar engine reading PSUM directly, and accumulates per-token partial
    sums-of-exp via the activation accum_out path.
  - host combines: logZ = log(sum_c s_c), target logit is an exact fp64
    dot on host (O(T*D), 0.001% of the FLOPs), loss = mean(logZ - tgt).

No max-subtraction is needed: logits are bounded by ||h_t||*||w_v|| <= ~36
for this problem family (hidden ~N(0,1), weight ~N(0,1/D)), far below fp32
exp overflow (~88), and sum-of-exp over 16k terms stays ~1e5 << fp32 max.
"""

import sys

import numpy as np

if "/opt/trn_rl_repo" not in sys.path:
    sys.path.insert(0, "/opt/trn_rl_repo")

import ml_dtypes

B, S, D, V = 2, 1024, 1024, 128000
NCORES = 8
VS = V // NCORES          # vocab shard per core
T = B * S                 # tokens
P = 128                   # partitions
KC = D // P               # contraction chunks
MT = T // P               # token tiles (psum partition dim)
NT = 512                  # vocab tile (psum free dim, one bank fp32)
N_TILES = (VS + NT - 1) // NT
IGNORE_INDEX = -100
WSCALE = 32.0             # fp8 mode: host multiplies weight by this,
                          # the exp activation divides it back out
MODE = "fp8"              # "bf16" or "fp8"

_CACHE = {}


def _build_nc(t=T, vs=VS, d=D, mode=MODE):
    import concourse.tile as tile
    from concourse import bacc, mybir

    kc = d // P
    mt = t // P
    n_tiles = (vs + NT - 1) // NT
    in_dt = mybir.dt.float8e4 if mode == "fp8" else mybir.dt.bfloat16
    exp_scale = (1.0 / WSCALE) if mode == "fp8" else 1.0

    nc = bacc.Bacc("TRN2", target_bir_lowering=False, debug=False,
                   num_devices=NCORES)
    h_dram = nc.declare_dram_parameter("h", [d, t], in_dt, isOutput=False)
    w_dram = nc.declare_dram_parameter("w", [d, vs], in_dt, isOutput=False)
    s_dram = nc.declare_dram_parameter("s_out", [P, mt], mybir.dt.float32,
                                       isOutput=True)

    with tile.TileContext(nc) as tc:
        with (
            tc.tile_pool(name="hp", bufs=1) as hp,
            tc.tile_pool(name="wp", bufs=3) as wp,
            tc.tile_pool(name="pp", bufs=(2 if mode == "fp8" else 6),
                         space="PSUM") as pp,
            tc.tile_pool(name="ep", bufs=3) as ep,
            tc.tile_pool(name="sp", bufs=1) as sp,
            tc.tile_pool(name="fp", bufs=1) as fp,
        ):
            h_sb = hp.tile([P, kc, t], in_dt, name="h_sb")
            for kh in range(2):
                nc.sync.dma_start(
                    out=h_sb[:, kh * (kc // 2):(kh + 1) * (kc // 2), :],
                    in_=h_dram.rearrange("(k p) t -> p k t", p=P)
                    [:, kh * (kc // 2):(kh + 1) * (kc // 2), :])
            if mode == "fp8":
                # 16000 = 8 groups x (NG=4 banks) x (NW=500 vocab each).
                # One activation spans a whole 4-bank psum group (2000
                # elems of the SAME token tile) to amortize the ~352-cycle
                # ACTIVATE fixed cost; accum_out then sums the full group.
                NW, NG = 500, 4
                n_grps = vs // (NW * NG)
                assert vs == n_grps * NW * NG
                s_parts = sp.tile([P, mt, n_grps], mybir.dt.float32,
                                  name="s_parts")
                h_dr = h_sb.rearrange("p (c j) t -> p c j t", j=2)
                for ng in range(n_grps):
                    n0 = ng * NW * NG
                    w_sb = wp.tile([P, kc, NG, NW], in_dt, name="w_sb")
                    src = w_dram[:, n0:n0 + NW * NG].rearrange(
                        "(k p) (g n) -> p k g n", p=P, g=NG)
                    for kh in range(2):
                        ks = slice(kh * (kc // 2), (kh + 1) * (kc // 2))
                        nc.sync.dma_start(out=w_sb[:, ks], in_=src[:, ks])
                    w_dr = w_sb.rearrange("p (c j) g n -> p c j g n", j=2)
                    for m in range(mt):
                        # inner dim padded to 512 floats = 2048 B so every
                        # gi slab starts on a PSUM bank boundary
                        pt4 = pp.tile([P, NG, 512], mybir.dt.float32,
                                      name="pt4")
                        for gi in range(NG):
                            for c in range(kc // 2):
                                nc.tensor.matmul(
                                    pt4[:, gi, :NW],
                                    lhsT=h_dr[:, c, :, m * P:(m + 1) * P],
                                    rhs=w_dr[:, c, :, gi, :],
                                    start=(c == 0),
                                    stop=(c == kc // 2 - 1),
                                    perf_mode=mybir.MatmulPerfMode.DoubleRow,
                                )
                        ex = ep.tile([P, NG, NW], mybir.dt.bfloat16,
                                     name="ex")
                        nc.scalar.activation(
                            out=ex[:],
                            in_=pt4[:, :, :NW],
                            func=mybir.ActivationFunctionType.Exp,
                            scale=exp_scale,
                            accum_out=s_parts[:, m, ng:ng + 1],
                        )
                n_tiles = n_grps
            else:
                s_parts = sp.tile([P, mt, n_tiles], mybir.dt.float32,
                                  name="s_parts")
                for n in range(n_tiles):
                    n0 = n * NT
                    nsz = min(NT, vs - n0)
                    w_sb = wp.tile([P, kc, NT], in_dt, name="w_sb")
                    nc.sync.dma_start(
                        out=w_sb[:, :, :nsz],
                        in_=w_dram[:, n0:n0 + nsz].rearrange(
                            "(k p) n -> p k n", p=P),
                    )
                    for m in range(mt):
                        pt = pp.tile([P, NT], mybir.dt.float32, name="pt")
                        for k in range(kc):
                            nc.tensor.matmul(
                                pt[:, :nsz],
                                lhsT=h_sb[:, k, m * P:(m + 1) * P],
                                rhs=w_sb[:, k, :nsz],
                                start=(k == 0),
                                stop=(k == kc - 1),
                            )
                        ex = ep.tile([P, NT], mybir.dt.bfloat16, name="ex")
                        nc.scalar.activation(
                            out=ex[:, :nsz],
                            in_=pt[:, :nsz],
                            func=mybir.ActivationFunctionType.Exp,
                            scale=exp_scale,
                            accum_out=s_parts[:, m, n:n + 1],
                        )
            s_fin = fp.tile([P, mt], mybir.dt.float32, name="s_fin")
            nc.vector.tensor_reduce(
                out=s_fin[:],
                in_=s_parts[:],
                axis=mybir.AxisListType.X,
                op=mybir.AluOpType.add,
            )
            nc.sync.dma_start(out=s_dram[:, :], in_=s_fin[:])
    nc.compile()
    return nc


def _get_nc():
    if "nc" not in _CACHE:
        _CACHE["nc"] = _build_nc()
    return _CACHE["nc"]


def _device_sumexp(hidden_td, weight, trace=False, trace_cores=None):
    """hidden_td: [T, D] fp32; weight: [V, D] fp32.

    Returns (s [T] float64 = sum_v exp(logits), BassKernelResults)."""
    from concourse import mybir
    from concourse.bass_utils import run_bass_kernel_spmd

    nc = _get_nc()
    if MODE == "fp8":
        in_np_dt = mybir.dt.np(mybir.dt.float8e4)
        wmul = WSCALE
    else:
        in_np_dt = ml_dtypes.bfloat16
        wmul = 1.0
    h_bf = np.ascontiguousarray(hidden_td.astype(in_np_dt).T)  # [D, T]
    in_maps = []
    for c in range(NCORES):
        w_shard = weight[c * VS:(c + 1) * VS, :]               # [VS, D]
        w_bf = np.ascontiguousarray(
            (w_shard * wmul).astype(in_np_dt).T)               # [D, VS]
        in_maps.append({"h": h_bf, "w": w_bf})
    res = run_bass_kernel_spmd(nc, in_maps, list(range(NCORES)),
                               trace=trace, trace_cores=trace_cores)
    s = np.zeros(T, dtype=np.float64)
    for c in range(NCORES):
        out = np.asarray(res.results[c]["s_out"], dtype=np.float64)  # [P, MT]
        s += out.T.reshape(T)     # token index = m*128 + p
    return s, res


def kernel(hidden, weight, targets):
    hidden_td = np.ascontiguousarray(
        np.asarray(hidden, dtype=np.float32).reshape(T, D))
    weight = np.asarray(weight, dtype=np.float32)
    tflat = np.asarray(targets).reshape(T)

    s, _ = _device_sumexp(hidden_td, weight)
    logZ = np.log(s)

    mask = tflat != IGNORE_INDEX
    safe_t = np.where(mask, tflat, 0).astype(np.int64)
    wg = weight[safe_t, :].astype(np.float64)
    tgt = np.einsum("td,td->t", hidden_td.astype(np.float64), wg)
    nll = np.where(mask, logZ - tgt, 0.0)
    n = float(mask.sum())
    total = float(nll.sum())
    loss = total if n == 0.0 else total / max(n, 1.0)
    return np.array(loss, dtype=np.float32)


# revision 12
# speedup vs baseline: 1.9424x; 1.0020x over previous
"""LinearCrossEntropyLoss kernel for 8 Trainium2 NeuronCores.

Strategy (tensor-parallel over vocab):
  - weight [V=128000, D=1024] is sharded over 8 cores: 16000 vocab rows each.
  - each core computes logits[t, v_shard] = hidden @ w_shard^T in bf16 tiles
    (tokens on PSUM partitions, vocab on free dim), applies exp on the
    scalar engine reading PSUM directly, and accumulates per-token partial
    sums-of-exp via the activation accum_out path.
  - host combines: logZ = log(sum_c s_c), target logit is an exact fp64
    dot on host (O(T*D), 0.001% of the FLOPs), loss = mean(logZ - tgt).

No max-subtraction is needed: logits are bounded by ||h_t||*||w_v|| <= ~36
for this problem family (hidden ~N(0,1), weight ~N(0,1/D)), far below fp32
exp overflow (~88), and sum-of-exp over 16k terms stays ~1e5 << fp32 max.
"""

import sys

import numpy as np

if "/opt/trn_rl_repo" not in sys.path:
    sys.path.insert(0, "/opt/trn_rl_repo")

import ml_dtypes

B, S, D, V = 2, 1024, 1024, 128000
NCORES = 8
VS = V // NCORES          # vocab shard per core
T = B * S                 # tokens
P = 128                   # partitions
KC = D // P               # contraction chunks
MT = T // P               # token tiles (psum partition dim)
NT = 512                  # vocab tile (psum free dim, one bank fp32)
N_TILES = (VS + NT - 1) // NT
IGNORE_INDEX = -100
WSCALE = 32.0             # fp8 mode: host multiplies weight by this,
                          # the exp activation divides it back out
MODE = "fp8"              # "bf16" or "fp8"

_CACHE = {}


def _build_nc(t=T, vs=VS, d=D, mode=MODE):
    import concourse.tile as tile
    from concourse import bacc, mybir

    kc = d // P
    mt = t // P
    n_tiles = (vs + NT - 1) // NT
    in_dt = mybir.dt.float8e4 if mode == "fp8" else mybir.dt.bfloat16
    exp_scale = (1.0 / WSCALE) if mode == "fp8" else 1.0

    nc = bacc.Bacc("TRN2", target_bir_lowering=False, debug=False,
                   num_devices=NCORES)
    h_dram = nc.declare_dram_parameter("h", [d, t], in_dt, isOutput=False)
    w_dram = nc.declare_dram_parameter("w", [d, vs], in_dt, isOutput=False)
    s_dram = nc.declare_dram_parameter("s_out", [P, mt], mybir.dt.float32,
                                       isOutput=True)

    with tile.TileContext(nc) as tc:
        with (
            tc.tile_pool(name="hp", bufs=1) as hp,
            tc.tile_pool(name="wp", bufs=4) as wp,
            tc.tile_pool(name="pp", bufs=(2 if mode == "fp8" else 6),
                         space="PSUM") as pp,
            tc.tile_pool(name="ep", bufs=3) as ep,
            tc.tile_pool(name="sp", bufs=1) as sp,
            tc.tile_pool(name="fp", bufs=1) as fp,
        ):
            h_sb = hp.tile([P, kc, t], in_dt, name="h_sb")
            for kh in range(2):
                # gpsimd queue: separate HWDGE ring from the w loads on sync
                nc.gpsimd.dma_start(
                    out=h_sb[:, kh * (kc // 2):(kh + 1) * (kc // 2), :],
                    in_=h_dram.rearrange("(k p) t -> p k t", p=P)
                    [:, kh * (kc // 2):(kh + 1) * (kc // 2), :])
            if mode == "fp8":
                # 16000 = 8 groups x (NG=4 banks) x (NW=500 vocab each).
                # One activation spans a whole 4-bank psum group (2000
                # elems of the SAME token tile) to amortize the ~352-cycle
                # ACTIVATE fixed cost; accum_out then sums the full group.
                NW, NG = 500, 4
                n_grps = vs // (NW * NG)
                assert vs == n_grps * NW * NG
                s_parts = sp.tile([P, mt, n_grps], mybir.dt.float32,
                                  name="s_parts")
                h_dr = h_sb.rearrange("p (c j) t -> p c j t", j=2)
                for ng in range(n_grps):
                    n0 = ng * NW * NG
                    w_sb = wp.tile([P, kc, NG, NW], in_dt, name="w_sb")
                    src = w_dram[:, n0:n0 + NW * NG].rearrange(
                        "(k p) (g n) -> p k g n", p=P, g=NG)
                    for kh in range(4):
                        ks = slice(kh * (kc // 4), (kh + 1) * (kc // 4))
                        nc.sync.dma_start(out=w_sb[:, ks], in_=src[:, ks])
                    w_dr = w_sb.rearrange("p (c j) g n -> p c j g n", j=2)
                    for m in range(mt):
                        # inner dim padded to 512 floats = 2048 B so every
                        # gi slab starts on a PSUM bank boundary
                        pt4 = pp.tile([P, NG, 512], mybir.dt.float32,
                                      name="pt4")
                        # c outer / gi inner: 4 consecutive matmuls share
                        # the stationary operand -> 1 LDWEIGHTS per 4
                        for c in range(kc // 2):
                            for gi in range(NG):
                                nc.tensor.matmul(
                                    pt4[:, gi, :NW],
                                    lhsT=h_dr[:, c, :, m * P:(m + 1) * P],
                                    rhs=w_dr[:, c, :, gi, :],
                                    start=(c == 0),
                                    stop=(c == kc // 2 - 1),
                                    perf_mode=mybir.MatmulPerfMode.DoubleRow,
                                )
                        ex = ep.tile([P, NG, NW], mybir.dt.bfloat16,
                                     name="ex")
                        nc.scalar.activation(
                            out=ex[:],
                            in_=pt4[:, :, :NW],
                            func=mybir.ActivationFunctionType.Exp,
                            scale=exp_scale,
                            accum_out=s_parts[:, m, ng:ng + 1],
                        )
                n_tiles = n_grps
            else:
                s_parts = sp.tile([P, mt, n_tiles], mybir.dt.float32,
                                  name="s_parts")
                for n in range(n_tiles):
                    n0 = n * NT
                    nsz = min(NT, vs - n0)
                    w_sb = wp.tile([P, kc, NT], in_dt, name="w_sb")
                    nc.sync.dma_start(
                        out=w_sb[:, :, :nsz],
                        in_=w_dram[:, n0:n0 + nsz].rearrange(
                            "(k p) n -> p k n", p=P),
                    )
                    for m in range(mt):
                        pt = pp.tile([P, NT], mybir.dt.float32, name="pt")
                        for k in range(kc):
                            nc.tensor.matmul(
                                pt[:, :nsz],
                                lhsT=h_sb[:, k, m * P:(m + 1) * P],
                                rhs=w_sb[:, k, :nsz],
                                start=(k == 0),
                                stop=(k == kc - 1),
                            )
                        ex = ep.tile([P, NT], mybir.dt.bfloat16, name="ex")
                        nc.scalar.activation(
                            out=ex[:, :nsz],
                            in_=pt[:, :nsz],
                            func=mybir.ActivationFunctionType.Exp,
                            scale=exp_scale,
                            accum_out=s_parts[:, m, n:n + 1],
                        )
            s_fin = fp.tile([P, mt], mybir.dt.float32, name="s_fin")
            nc.vector.tensor_reduce(
                out=s_fin[:],
                in_=s_parts[:],
                axis=mybir.AxisListType.X,
                op=mybir.AluOpType.add,
            )
            nc.sync.dma_start(out=s_dram[:, :], in_=s_fin[:])
    nc.compile()
    return nc


def _get_nc():
    if "nc" not in _CACHE:
        _CACHE["nc"] = _build_nc()
    return _CACHE["nc"]


def _device_sumexp(hidden_td, weight, trace=False, trace_cores=None):
    """hidden_td: [T, D] fp32; weight: [V, D] fp32.

    Returns (s [T] float64 = sum_v exp(logits), BassKernelResults)."""
    from concourse import mybir
    from concourse.bass_utils import run_bass_kernel_spmd

    nc = _get_nc()
    if MODE == "fp8":
        in_np_dt = mybir.dt.np(mybir.dt.float8e4)
        wmul = WSCALE
    else:
        in_np_dt = ml_dtypes.bfloat16
        wmul = 1.0
    h_bf = np.ascontiguousarray(hidden_td.astype(in_np_dt).T)  # [D, T]
    in_maps = []
    for c in range(NCORES):
        w_shard = weight[c * VS:(c + 1) * VS, :]               # [VS, D]
        w_bf = np.ascontiguousarray(
            (w_shard * wmul).astype(in_np_dt).T)               # [D, VS]
        in_maps.append({"h": h_bf, "w": w_bf})
    res = run_bass_kernel_spmd(nc, in_maps, list(range(NCORES)),
                               trace=trace, trace_cores=trace_cores)
    s = np.zeros(T, dtype=np.float64)
    for c in range(NCORES):
        out = np.asarray(res.results[c]["s_out"], dtype=np.float64)  # [P, MT]
        s += out.T.reshape(T)     # token index = m*128 + p
    return s, res


def kernel(hidden, weight, targets):
    hidden_td = np.ascontiguousarray(
        np.asarray(hidden, dtype=np.float32).reshape(T, D))
    weight = np.asarray(weight, dtype=np.float32)
    tflat = np.asarray(targets).reshape(T)

    s, _ = _device_sumexp(hidden_td, weight)
    logZ = np.log(s)

    mask = tflat != IGNORE_INDEX
    safe_t = np.where(mask, tflat, 0).astype(np.int64)
    wg = weight[safe_t, :].astype(np.float64)
    tgt = np.einsum("td,td->t", hidden_td.astype(np.float64), wg)
    nll = np.where(mask, logZ - tgt, 0.0)
    n = float(mask.sum())
    total = float(nll.sum())
    loss = total if n == 0.0 else total / max(n, 1.0)
    return np.array(loss, dtype=np.float32)


# revision 13
# speedup vs baseline: 1.9656x; 1.0119x over previous
"""LinearCrossEntropyLoss kernel for 8 Trainium2 NeuronCores.

Strategy (tensor-parallel over vocab):
  - weight [V=128000, D=1024] is sharded over 8 cores: 16000 vocab rows each.
  - each core computes logits[t, v_shard] = hidden @ w_shard^T in bf16 tiles
    (tokens on PSUM partitions, vocab on free dim), applies exp on the
    scalar engine reading PSUM directly, and accumulates per-token partial
    sums-of-exp via the activation accum_out path.
  - host combines: logZ = log(sum_c s_c), target logit is an exact fp64
    dot on host (O(T*D), 0.001% of the FLOPs), loss = mean(logZ - tgt).

No max-subtraction is needed: logits are bounded by ||h_t||*||w_v|| <= ~36
for this problem family (hidden ~N(0,1), weight ~N(0,1/D)), far below fp32
exp overflow (~88), and sum-of-exp over 16k terms stays ~1e5 << fp32 max.
"""

import sys

import numpy as np

if "/opt/trn_rl_repo" not in sys.path:
    sys.path.insert(0, "/opt/trn_rl_repo")

import ml_dtypes

B, S, D, V = 2, 1024, 1024, 128000
NCORES = 8
VS = V // NCORES          # vocab shard per core
T = B * S                 # tokens
P = 128                   # partitions
KC = D // P               # contraction chunks
MT = T // P               # token tiles (psum partition dim)
NT = 512                  # vocab tile (psum free dim, one bank fp32)
N_TILES = (VS + NT - 1) // NT
IGNORE_INDEX = -100
WSCALE = 32.0             # fp8 mode: host multiplies weight by this,
                          # the exp activation divides it back out
MODE = "fp8"              # "bf16" or "fp8"

_CACHE = {}


def _build_nc(t=T, vs=VS, d=D, mode=MODE):
    import concourse.tile as tile
    from concourse import bacc, mybir

    kc = d // P
    mt = t // P
    n_tiles = (vs + NT - 1) // NT
    in_dt = mybir.dt.float8e4 if mode == "fp8" else mybir.dt.bfloat16
    exp_scale = (1.0 / WSCALE) if mode == "fp8" else 1.0

    nc = bacc.Bacc("TRN2", target_bir_lowering=False, debug=False,
                   num_devices=NCORES)
    h_dram = nc.declare_dram_parameter("h", [d, t], in_dt, isOutput=False)
    w_dram = nc.declare_dram_parameter("w", [d, vs], in_dt, isOutput=False)
    s_dram = nc.declare_dram_parameter("s_out", [P, mt], mybir.dt.float32,
                                       isOutput=True)

    with tile.TileContext(nc) as tc:
        with (
            tc.tile_pool(name="hp", bufs=1) as hp,
            tc.tile_pool(name="wp", bufs=4) as wp,
            tc.tile_pool(name="pp", bufs=(4 if mode == "fp8" else 6),
                         space="PSUM") as pp,
            tc.tile_pool(name="ep", bufs=3) as ep,
            tc.tile_pool(name="sp", bufs=1) as sp,
            tc.tile_pool(name="fp", bufs=1) as fp,
        ):
            h_sb = hp.tile([P, kc, t], in_dt, name="h_sb")
            for kh in range(2):
                # scalar queue: separate HWDGE ring from the w loads on sync
                nc.scalar.dma_start(
                    out=h_sb[:, kh * (kc // 2):(kh + 1) * (kc // 2), :],
                    in_=h_dram.rearrange("(k p) t -> p k t", p=P)
                    [:, kh * (kc // 2):(kh + 1) * (kc // 2), :])
            if mode == "fp8":
                # 16000 = 8 groups x (NG=4 banks) x (NW=500 vocab each).
                # One activation spans a whole 4-bank psum group (2000
                # elems of the SAME token tile) to amortize the ~352-cycle
                # ACTIVATE fixed cost; accum_out then sums the full group.
                NW, NG = 500, 2
                n_grps = vs // (NW * NG)
                assert vs == n_grps * NW * NG
                s_parts = sp.tile([P, mt, n_grps], mybir.dt.float32,
                                  name="s_parts")
                h_dr = h_sb.rearrange("p (c j) t -> p c j t", j=2)
                for ng in range(n_grps):
                    n0 = ng * NW * NG
                    w_sb = wp.tile([P, kc, NG, NW], in_dt, name="w_sb")
                    src = w_dram[:, n0:n0 + NW * NG].rearrange(
                        "(k p) (g n) -> p k g n", p=P, g=NG)
                    for kh in range(4):
                        ks = slice(kh * (kc // 4), (kh + 1) * (kc // 4))
                        nc.sync.dma_start(out=w_sb[:, ks], in_=src[:, ks])
                    w_dr = w_sb.rearrange("p (c j) g n -> p c j g n", j=2)
                    for m in range(mt):
                        # inner dim padded to 512 floats = 2048 B so every
                        # gi slab starts on a PSUM bank boundary
                        pt4 = pp.tile([P, NG, 512], mybir.dt.float32,
                                      name="pt4")
                        # c outer / gi inner: 4 consecutive matmuls share
                        # the stationary operand -> 1 LDWEIGHTS per 4
                        for c in range(kc // 2):
                            for gi in range(NG):
                                nc.tensor.matmul(
                                    pt4[:, gi, :NW],
                                    lhsT=h_dr[:, c, :, m * P:(m + 1) * P],
                                    rhs=w_dr[:, c, :, gi, :],
                                    start=(c == 0),
                                    stop=(c == kc // 2 - 1),
                                    perf_mode=mybir.MatmulPerfMode.DoubleRow,
                                )
                        ex = ep.tile([P, NG, NW], mybir.dt.bfloat16,
                                     name="ex")
                        nc.scalar.activation(
                            out=ex[:],
                            in_=pt4[:, :, :NW],
                            func=mybir.ActivationFunctionType.Exp,
                            scale=exp_scale,
                            accum_out=s_parts[:, m, ng:ng + 1],
                        )
                n_tiles = n_grps
            else:
                s_parts = sp.tile([P, mt, n_tiles], mybir.dt.float32,
                                  name="s_parts")
                for n in range(n_tiles):
                    n0 = n * NT
                    nsz = min(NT, vs - n0)
                    w_sb = wp.tile([P, kc, NT], in_dt, name="w_sb")
                    nc.sync.dma_start(
                        out=w_sb[:, :, :nsz],
                        in_=w_dram[:, n0:n0 + nsz].rearrange(
                            "(k p) n -> p k n", p=P),
                    )
                    for m in range(mt):
                        pt = pp.tile([P, NT], mybir.dt.float32, name="pt")
                        for k in range(kc):
                            nc.tensor.matmul(
                                pt[:, :nsz],
                                lhsT=h_sb[:, k, m * P:(m + 1) * P],
                                rhs=w_sb[:, k, :nsz],
                                start=(k == 0),
                                stop=(k == kc - 1),
                            )
                        ex = ep.tile([P, NT], mybir.dt.bfloat16, name="ex")
                        nc.scalar.activation(
                            out=ex[:, :nsz],
                            in_=pt[:, :nsz],
                            func=mybir.ActivationFunctionType.Exp,
                            scale=exp_scale,
                            accum_out=s_parts[:, m, n:n + 1],
                        )
            s_fin = fp.tile([P, mt], mybir.dt.float32, name="s_fin")
            nc.vector.tensor_reduce(
                out=s_fin[:],
                in_=s_parts[:],
                axis=mybir.AxisListType.X,
                op=mybir.AluOpType.add,
            )
            nc.sync.dma_start(out=s_dram[:, :], in_=s_fin[:])
    nc.compile()
    return nc


def _get_nc():
    if "nc" not in _CACHE:
        _CACHE["nc"] = _build_nc()
    return _CACHE["nc"]


def _device_sumexp(hidden_td, weight, trace=False, trace_cores=None):
    """hidden_td: [T, D] fp32; weight: [V, D] fp32.

    Returns (s [T] float64 = sum_v exp(logits), BassKernelResults)."""
    from concourse import mybir
    from concourse.bass_utils import run_bass_kernel_spmd

    nc = _get_nc()
    if MODE == "fp8":
        in_np_dt = mybir.dt.np(mybir.dt.float8e4)
        wmul = WSCALE
    else:
        in_np_dt = ml_dtypes.bfloat16
        wmul = 1.0
    h_bf = np.ascontiguousarray(hidden_td.astype(in_np_dt).T)  # [D, T]
    in_maps = []
    for c in range(NCORES):
        w_shard = weight[c * VS:(c + 1) * VS, :]               # [VS, D]
        w_bf = np.ascontiguousarray(
            (w_shard * wmul).astype(in_np_dt).T)               # [D, VS]
        in_maps.append({"h": h_bf, "w": w_bf})
    res = run_bass_kernel_spmd(nc, in_maps, list(range(NCORES)),
                               trace=trace, trace_cores=trace_cores)
    s = np.zeros(T, dtype=np.float64)
    for c in range(NCORES):
        out = np.asarray(res.results[c]["s_out"], dtype=np.float64)  # [P, MT]
        s += out.T.reshape(T)     # token index = m*128 + p
    return s, res


def kernel(hidden, weight, targets):
    hidden_td = np.ascontiguousarray(
        np.asarray(hidden, dtype=np.float32).reshape(T, D))
    weight = np.asarray(weight, dtype=np.float32)
    tflat = np.asarray(targets).reshape(T)

    s, _ = _device_sumexp(hidden_td, weight)
    logZ = np.log(s)

    mask = tflat != IGNORE_INDEX
    safe_t = np.where(mask, tflat, 0).astype(np.int64)
    wg = weight[safe_t, :].astype(np.float64)
    tgt = np.einsum("td,td->t", hidden_td.astype(np.float64), wg)
    nll = np.where(mask, logZ - tgt, 0.0)
    n = float(mask.sum())
    total = float(nll.sum())
    loss = total if n == 0.0 else total / max(n, 1.0)
    return np.array(loss, dtype=np.float32)
